# revision 23
# baseline (speedup 1.0000x reference)
"""Trainium2 Bass kernel for nn_DoublyEquivariantOrbitalLayer.

Math (per spin s, walker b):
  U[p,o]   = xs[p,:] @ W1[s]          (W1 = W_orb[s,:D,:])
  V[i,o]   = xs[i,:] @ W2[s]          (W2 = W_orb[s,D:,:])
  dist2[i,ion,o] = sum_{dA,dB} res[i,ion,dA]*res[i,ion,dB]*G[s,ion,dA,dB,o]
      where G[s,ion,dA,dB,o] = sum_e W_env[s,ion,dA,o,e]*W_env[s,ion,dB,o,e]
  env[i,o] = sum_ion w_ion[s,ion] * exp(-sqrt(dist2[i,ion,o]))
  out[p,i,o] = (U[p,o] + V[i,o] + b_orb[s,o]) * env[i,o]

Device layout: partitions = (4 walkers x 32 orbitals), free = (pair j, p, i).
Two groups (2x4 walkers, same spin) are fused per iteration to amortize
per-instruction overheads. All broadcast operands are small tiles read via
stride-0 access patterns, so the 268MB output needs exactly two full DVE
passes (add, then mul). sqrt is computed as exp(0.5*ln(x)) so the whole env
chain stays in one ACT table set (natural_log_exp_and_others) -- the
activation-table assigner is nudged via a patched get_activation_tables so
ln/exp resolve to the combined set (no ~2.7us table reloads).

Sharding: data-parallel over walkers, 128 walkers/core on 8 cores; params are
tiny and folded into precomputed constants host-side.
"""

import sys

sys.path.insert(0, "/opt/trn_rl_repo")

import functools
import numpy as np
from contextlib import ExitStack

import concourse.bacc as bacc
import concourse.tile as tile
from concourse import mybir
from concourse.bass_utils import run_bass_kernel_spmd

# ---- patch the activation-table chooser: make ln/exp resolve to the combined
# natural_log_exp_and_others set (greedy first-match would otherwise alternate
# exp_and_others / natural_log and reload tables every group).
import concourse.hw_specs as _hw_specs
import concourse.bacc as _bacc_mod

_orig_get_tables = _hw_specs.get_activation_tables


@functools.cache
def _patched_get_tables(module_arch):
    tabs = dict(_orig_get_tables(module_arch))
    af = mybir.ActivationFunctionType
    combined = "natural_log_exp_and_others"
    if combined in tabs:
        out = {}
        for name, fns in tabs.items():
            if name != combined:
                fns = fns - {af.Exp, af.Ln}
            out[name] = fns
        return out
    return tabs


import os as _os
if not int(_os.environ.get("NO_ACT_PATCH", "0")):
    _hw_specs.get_activation_tables = _patched_get_tables
    _bacc_mod.get_activation_tables = _patched_get_tables

# Problem dims (hardcoded per spec)
B, NELEC, D, NION, SPATIAL, NORB = 1024, 64, 32, 16, 3, 32
NSPIN = 2
NE = NELEC // NSPIN  # 32
NCORES = 8
WC = B // NCORES     # 128 walkers per core
NWG = WC // 4        # 32 walker-groups of 4
NGRP = NWG * NSPIN   # 64 groups per core
NPAIR = NGRP // 2    # 32 fused pairs (both groups of a pair share the spin)
DT = mybir.dt.float32

# dd8 pair order for the quadratic form: diag, (01),(12),(02), 2x zero-pad
_DD6 = [(0, 0), (1, 1), (2, 2), (0, 1), (1, 2), (0, 2)]

_NC_CACHE = None


def _build_nc(repeat=1, hw_loop=False):
    nc = bacc.Bacc(None, target_bir_lowering=False, debug=True)

    ing = nc.dram_tensor("ing", [NPAIR, 128, 160], DT, kind="ExternalInput")
    blkw = nc.dram_tensor("blkw", [NSPIN, 2, 128, 128], DT, kind="ExternalInput")
    # gqp: per (spin, quad) a k=128 lhsT with rows (ion, dd8); rows outside
    # the quad are zero, so every matmul operand sits at partition base 0.
    gqp = nc.dram_tensor("gqp", [128, 4 * NSPIN, 128], DT, kind="ExternalInput")
    wsel = nc.dram_tensor("wsel", [NSPIN, 4, 128, 32], DT, kind="ExternalInput")
    borb = nc.dram_tensor("borb", [128, NSPIN], DT, kind="ExternalInput")
    i128 = nc.dram_tensor("i128", [128, 128], DT, kind="ExternalInput")
    out = nc.dram_tensor("out", [NPAIR, 128, 2048], DT, kind="ExternalOutput")

    with tile.TileContext(nc) as tc, ExitStack() as ctx:
        consts = ctx.enter_context(tc.tile_pool(name="consts", bufs=1))
        inp = ctx.enter_context(tc.tile_pool(name="inp", bufs=4))
        work = ctx.enter_context(tc.tile_pool(name="work", bufs=3))
        big = ctx.enter_context(tc.tile_pool(name="big", bufs=4))
        ps1 = ctx.enter_context(tc.tile_pool(name="ps1", bufs=1, space="PSUM"))
        ps2 = ctx.enter_context(tc.tile_pool(name="ps2", bufs=2, space="PSUM"))

        sb_blkw = consts.tile([128, 2 * NSPIN, 128], DT)
        nc.sync.dma_start(out=sb_blkw, in_=blkw.rearrange("s u k m -> k (s u) m"))
        sb_gqp = consts.tile([128, 4 * NSPIN, 128], DT)
        nc.sync.dma_start(out=sb_gqp, in_=gqp[:, :, :])
        sb_wsel = consts.tile([128, 4 * NSPIN, 32], DT)
        nc.sync.dma_start(out=sb_wsel, in_=wsel.rearrange("s q k m -> k (s q) m"))
        sb_borb = consts.tile([128, NSPIN], DT)
        nc.sync.dma_start(out=sb_borb, in_=borb[:, :])
        sb_i128 = consts.tile([128, 128], DT)
        nc.sync.dma_start(out=sb_i128, in_=i128[:, :])
        sb_eps = consts.tile([128, 1], DT)
        nc.vector.memset(sb_eps, 1e-12)

        loop_ctx = tc.For_i(0, repeat, 1) if hw_loop else None
        if loop_ctx is not None:
            ctx.enter_context(loop_ctx)
        for rep in range(1 if hw_loop else repeat):
          for gp in range(NPAIR):
            s = (2 * gp) // NWG

            sb_in = inp.tile([128, 160], DT)
            nc.sync.dma_start(out=sb_in, in_=ing[gp, :, :])
            xt2 = sb_in[:, 0:64]                         # [(b,f), (j, e)]
            res = sb_in[:, 64:160].rearrange("r (j i d) -> r j i d",
                                             j=2, d=3)

            # U,V block-diag matmuls -> one PSUM bank [128, (uv, j, o)]
            uv_ps = ps1.tile([128, 128], DT)
            nc.tensor.matmul(uv_ps[:, 0:64], sb_blkw[:, 2 * s + 0, :],
                             xt2, start=True, stop=True)
            nc.tensor.matmul(uv_ps[:, 64:128], sb_blkw[:, 2 * s + 1, :],
                             xt2, start=True, stop=True)
            # U' = U + b_orb (per-partition bias), PSUM -> SBUF; V copied
            # out too so the uv PSUM bank frees early (keeps PE unblocked)
            sb_u2 = work.tile([128, 64], DT)
            nc.scalar.activation(sb_u2, uv_ps[:, 0:64],
                                 mybir.ActivationFunctionType.Identity,
                                 bias=sb_borb[:, s:s + 1])
            sb_v2 = work.tile([128, 64], DT)
            nc.scalar.copy(sb_v2, uv_ps[:, 64:128])

            # resq products, layout [(b,i), (j, ion, dd8)], pads zeroed
            sb_resq = work.tile([128, 2, 16, 8], DT)
            nc.vector.memset(sb_resq[:, :, :, 6:8], 0.0)
            nc.vector.tensor_tensor(sb_resq[:, :, :, 0:3], res, res,
                                    op=mybir.AluOpType.mult)
            nc.vector.tensor_tensor(sb_resq[:, :, :, 3:5], res[:, :, :, 0:2],
                                    res[:, :, :, 1:3], op=mybir.AluOpType.mult)
            nc.vector.tensor_tensor(sb_resq[:, :, :, 5:6], res[:, :, :, 0:1],
                                    res[:, :, :, 2:3], op=mybir.AluOpType.mult)

            # full transpose per group -> rqt [(ion,dd8), (j, b*i)], base 0
            rqt_ps = ps1.tile([128, 2, 128], DT)
            for j in range(2):
                nc.tensor.transpose(
                    rqt_ps[:, j, :],
                    sb_resq[:, j].rearrange("r i d -> r (i d)"), sb_i128)
            sb_rqt = work.tile([128, 2, 128], DT)
            nc.scalar.copy(sb_rqt, rqt_ps)

            # dist2 (transposed): per-quad k=128 matmuls with zero-padded
            # lhsT -- out [(ionl,o), (q, j, b, i)], 2 PSUM banks
            d2_ps = ps2.tile([128, 4, 2, 128], DT)
            rqt_full = sb_rqt.rearrange("r j n -> r (j n)")
            for q in range(4):
                nc.tensor.matmul(
                    d2_ps[:, q].rearrange("r j n -> r (j n)"),
                    sb_gqp[:, 4 * s + q, :], rqt_full,
                    start=True, stop=True)
            # env chain in one ACT table set: dist=exp(0.5*ln(d2)); exp(-dist)
            sb_lnd = big.tile([128, 1024], DT)
            nc.scalar.activation(sb_lnd, d2_ps.rearrange("r q j n -> r (q j n)"),
                                 mybir.ActivationFunctionType.Ln, bias=sb_eps)
            sb_dist = big.tile([128, 1024], DT)
            nc.scalar.activation(sb_dist, sb_lnd,
                                 mybir.ActivationFunctionType.Exp, scale=0.5)
            sb_expd = big.tile([128, 4, 2, 128], DT)
            nc.scalar.activation(sb_expd.rearrange("r q j n -> r (q j n)"),
                                 sb_dist,
                                 mybir.ActivationFunctionType.Exp, scale=-1.0)

            # env: accumulate ion quads -> [32 o, (j, b, i)]
            env_ps = ps1.tile([32, 2, 128], DT)
            for q in range(4):
                nc.tensor.matmul(env_ps.rearrange("o j n -> o (j n)"),
                                 sb_wsel[:, 4 * s + q, :],
                                 sb_expd[:, q].rearrange("r j n -> r (j n)"),
                                 start=(q == 0), stop=(q == 3))
            sb_envo = work.tile([32, 2, 128], DT)
            nc.scalar.copy(sb_envo, env_ps)
            # [o, (j, b, i)] -> [(b,i), (j, o)] via PE transpose ...
            envn_ps = ps1.tile([128, 2, 32], DT)
            for j in range(2):
                nc.tensor.transpose(envn_ps[:, j, :], sb_envo[:, j, :],
                                    sb_i128[0:32, 0:32])
            sb_envn = work.tile([128, 2, 32], DT)
            nc.scalar.copy(sb_envn, envn_ps)
            # ... then 32x32 block transpose: [(b,i), (j,o)] -> [(b,o), (j,i)]
            sb_envt = work.tile([128, 2, 32], DT)
            if int(__import__("os").environ.get("NO_VT", "0")):
                nc.vector.tensor_copy(
                    sb_envt.rearrange("r j i -> r (j i)"),
                    sb_envn.rearrange("r j o -> r (j o)"))
            else:
                nc.vector.transpose(sb_envt.rearrange("r j i -> r (j i)"),
                                    sb_envn.rearrange("r j o -> r (j o)"))

            # Final two DVE passes over [128, (j, p, i)]:
            #   S = V bcast_p + U' bcast_i ;  out = S * env bcast_p
            v_pair = sb_v2.rearrange("r (j o) -> r j o", j=2)
            u_pair = sb_u2.rearrange("r (j o) -> r j o", j=2)
            v_b = v_pair[:, :, None, :].broadcast_to([128, 2, 32, 32])
            u_b = u_pair[:, :, :, None].broadcast_to([128, 2, 32, 32])
            env_b = sb_envt[:, :, None, :].broadcast_to([128, 2, 32, 32])
            sb_s = big.tile([128, 2, 32, 32], DT)
            nc.vector.tensor_tensor(sb_s, v_b, u_b, op=mybir.AluOpType.add)
            sb_out = big.tile([128, 2, 32, 32], DT)
            nc.vector.tensor_tensor(sb_out, sb_s, env_b,
                                    op=mybir.AluOpType.mult)
            nc.sync.dma_start(out=out[gp, :, :],
                              in_=sb_out.rearrange("r j p i -> r (j p i)"))

    nc.compile()
    return nc


def _host_constants(W_orb, b_orb, W_env_dim, w_env_ion):
    W_orb = np.asarray(W_orb, np.float32)
    b_orb = np.asarray(b_orb, np.float32)
    W_env_dim = np.asarray(W_env_dim, np.float32)
    w_env_ion = np.asarray(w_env_ion, np.float32)

    W1 = W_orb[:, :D, :]   # [s, f, o]
    W2 = W_orb[:, D:, :]

    blkw = np.zeros((NSPIN, 2, 128, 128), np.float32)
    for s in range(NSPIN):
        for bb in range(4):
            sl = slice(32 * bb, 32 * bb + 32)
            blkw[s, 0, sl, sl] = W1[s]
            blkw[s, 1, sl, sl] = W2[s]

    # G6[s, ion, k, o] with pair order _DD6, off-diagonal doubled
    G = np.einsum("siaoe,siboe->siabo", W_env_dim, W_env_dim)
    G6 = np.empty((NSPIN, NION, 6, NORB), np.float32)
    for k, (dA, dB) in enumerate(_DD6):
        G6[:, :, k, :] = G[:, :, dA, dB, :] * (1.0 if dA == dB else 2.0)

    gq = np.zeros((NSPIN, 4, 32, 128), np.float32)
    for s in range(NSPIN):
        for q in range(4):
            for il in range(4):
                gq[s, q, 8 * il:8 * il + 6, 32 * il:32 * il + 32] = \
                    G6[s, 4 * q + il]
    # gqp[(ion,dd8), (s,q), (ionl,o)]: quad-q block at rows 32q, zeros else
    gqp = np.zeros((128, 4 * NSPIN, 128), np.float32)
    for s in range(NSPIN):
        for q in range(4):
            gqp[32 * q:32 * q + 32, 4 * s + q, :] = gq[s, q]

    wsel = np.zeros((NSPIN, 4, 128, 32), np.float32)
    eye = np.eye(32, dtype=np.float32)
    for s in range(NSPIN):
        for q in range(4):
            for il in range(4):
                wsel[s, q, 32 * il:32 * il + 32, :] = \
                    w_env_ion[s, 4 * q + il] * eye

    borb = np.zeros((128, NSPIN), np.float32)
    for s in range(NSPIN):
        borb[:, s] = np.tile(b_orb[s], 4)

    i128 = np.eye(128, dtype=np.float32)
    return dict(blkw=blkw, gqp=gqp, wsel=wsel, borb=borb, i128=i128)


def _host_inputs(x, r_ei):
    x = np.asarray(x, np.float32)
    r_ei = np.asarray(r_ei, np.float32)
    xr = x.reshape(NCORES, NWG, 4, NELEC, D)
    rr = r_ei.reshape(NCORES, NWG, 4, NELEC, NION, SPATIAL)
    ing = np.empty((NCORES, NGRP, 128, 80), np.float32)
    for s in range(NSPIN):
        xs = xr[:, :, :, 32 * s:32 * s + 32, :]        # c,wg,b,e,f
        ing[:, NWG * s:NWG * (s + 1), :, 0:32] = \
            xs.transpose(0, 1, 2, 4, 3).reshape(NCORES, NWG, 128, 32)
        rs = rr[:, :, :, 32 * s:32 * s + 32, :, :]     # c,wg,b,i,ion,d
        ing[:, NWG * s:NWG * (s + 1), :, 32:80] = \
            rs.reshape(NCORES, NWG, 128, 48)
    # pack pairs: [xt(j0) | xt(j1) | res(j0) | res(j1)] -> [NPAIR, 128, 160]
    ing2 = np.concatenate([ing[:, 0::2, :, 0:32], ing[:, 1::2, :, 0:32],
                           ing[:, 0::2, :, 32:80], ing[:, 1::2, :, 32:80]],
                          axis=3)
    return np.ascontiguousarray(ing2)


def kernel(x, r_ei, W_orb, b_orb, W_env_dim, w_env_ion):
    global _NC_CACHE
    if _NC_CACHE is None:
        _NC_CACHE = _build_nc()
    nc = _NC_CACHE

    consts = _host_constants(W_orb, b_orb, W_env_dim, w_env_ion)
    ing = _host_inputs(x, r_ei)

    in_maps = [dict(ing=ing[c], **consts) for c in range(NCORES)]
    res = run_bass_kernel_spmd(nc, in_maps, core_ids=list(range(NCORES)))

    arr = np.stack([res.results[c]["out"] for c in range(NCORES)])
    # [c, pair, (b,o), (j,p,i)] ; pair = (s, pl), walker = (c, 2*pl+j, b)
    arr = arr.reshape(NCORES, NSPIN, NWG // 2, 4, 32, 2, 32, 32)
    # dims: c s pl b o j p i -> s c pl j b p i o
    out = arr.transpose(1, 0, 2, 5, 3, 6, 7, 4).reshape(
        NSPIN, B, NE, NE, NORB)
    return np.ascontiguousarray(out)


if __name__ == "__main__":
    rng = np.random.default_rng(0)
    x = rng.standard_normal((B, NELEC, D), dtype=np.float32)
    r_ei = rng.standard_normal((B, NELEC, NION, SPATIAL), dtype=np.float32)
    W_orb = rng.standard_normal((NSPIN, 2 * D, NORB), dtype=np.float32)
    b_orb = rng.standard_normal((NSPIN, NORB), dtype=np.float32)
    W_env_dim = rng.standard_normal((NSPIN, NION, SPATIAL, NORB, SPATIAL),
                                    dtype=np.float32)
    w_env_ion = rng.standard_normal((NSPIN, NION), dtype=np.float32)
    o = kernel(x=x, r_ei=r_ei, W_orb=W_orb, b_orb=b_orb,
               W_env_dim=W_env_dim, w_env_ion=w_env_ion)
    print(o.shape, o.dtype)


# revision 24
# speedup vs baseline: 1.0909x; 1.0909x over previous
"""Trainium2 Bass kernel for nn_DoublyEquivariantOrbitalLayer.

Math (per spin s, walker b):
  U[p,o]   = xs[p,:] @ W1[s]          (W1 = W_orb[s,:D,:])
  V[i,o]   = xs[i,:] @ W2[s]          (W2 = W_orb[s,D:,:])
  dist2[i,ion,o] = sum_{dA,dB} res[i,ion,dA]*res[i,ion,dB]*G[s,ion,dA,dB,o]
      where G[s,ion,dA,dB,o] = sum_e W_env[s,ion,dA,o,e]*W_env[s,ion,dB,o,e]
  env[i,o] = sum_ion w_ion[s,ion] * exp(-sqrt(dist2[i,ion,o]))
  out[p,i,o] = (U[p,o] + V[i,o] + b_orb[s,o]) * env[i,o]

Device layout: partitions = (4 walkers x 32 orbitals), free = (pair j, p, i).
Two groups (2x4 walkers, same spin) are fused per iteration to amortize
per-instruction overheads. All broadcast operands are small tiles read via
stride-0 access patterns, so the 268MB output needs exactly two full DVE
passes (add, then mul). sqrt is computed as exp(0.5*ln(x)) so the whole env
chain stays in one ACT table set (natural_log_exp_and_others) -- the
activation-table assigner is nudged via a patched get_activation_tables so
ln/exp resolve to the combined set (no ~2.7us table reloads).

Sharding: data-parallel over walkers, 128 walkers/core on 8 cores; params are
tiny and folded into precomputed constants host-side.
"""

import sys

sys.path.insert(0, "/opt/trn_rl_repo")

import functools
import numpy as np
from contextlib import ExitStack

import concourse.bacc as bacc
import concourse.tile as tile
from concourse import mybir
from concourse.bass_utils import run_bass_kernel_spmd

# ---- patch the activation-table chooser: make ln/exp resolve to the combined
# natural_log_exp_and_others set (greedy first-match would otherwise alternate
# exp_and_others / natural_log and reload tables every group).
import concourse.hw_specs as _hw_specs
import concourse.bacc as _bacc_mod

_orig_get_tables = _hw_specs.get_activation_tables


@functools.cache
def _patched_get_tables(module_arch):
    tabs = dict(_orig_get_tables(module_arch))
    af = mybir.ActivationFunctionType
    combined = "natural_log_exp_and_others"
    if combined in tabs:
        out = {}
        for name, fns in tabs.items():
            if name != combined:
                fns = fns - {af.Exp, af.Ln}
            out[name] = fns
        return out
    return tabs


import os as _os
if not int(_os.environ.get("NO_ACT_PATCH", "0")):
    _hw_specs.get_activation_tables = _patched_get_tables
    _bacc_mod.get_activation_tables = _patched_get_tables

# Problem dims (hardcoded per spec)
B, NELEC, D, NION, SPATIAL, NORB = 1024, 64, 32, 16, 3, 32
NSPIN = 2
NE = NELEC // NSPIN  # 32
NCORES = 8
WC = B // NCORES     # 128 walkers per core
NWG = WC // 4        # 32 walker-groups of 4
NGRP = NWG * NSPIN   # 64 groups per core
NPAIR = NGRP // 2    # 32 fused pairs (both groups of a pair share the spin)
DT = mybir.dt.float32

# dd8 pair order for the quadratic form: diag, (01),(12),(02), 2x zero-pad
_DD6 = [(0, 0), (1, 1), (2, 2), (0, 1), (1, 2), (0, 2)]

_NC_CACHE = None


def _build_nc(repeat=1, hw_loop=False):
    nc = bacc.Bacc(None, target_bir_lowering=False, debug=True)

    ing = nc.dram_tensor("ing", [NPAIR, 128, 160], DT, kind="ExternalInput")
    blkw = nc.dram_tensor("blkw", [NSPIN, 2, 128, 128], DT, kind="ExternalInput")
    # gqp: per (spin, quad) a k=128 lhsT with rows (ion, dd8); rows outside
    # the quad are zero, so every matmul operand sits at partition base 0.
    gqp = nc.dram_tensor("gqp", [128, 4 * NSPIN, 128], DT, kind="ExternalInput")
    wsel = nc.dram_tensor("wsel", [NSPIN, 4, 128, 32], DT, kind="ExternalInput")
    borb = nc.dram_tensor("borb", [128, NSPIN], DT, kind="ExternalInput")
    i128 = nc.dram_tensor("i128", [128, 128], DT, kind="ExternalInput")
    out = nc.dram_tensor("out", [NPAIR, 128, 2048], DT, kind="ExternalOutput")

    with tile.TileContext(nc) as tc, ExitStack() as ctx:
        consts = ctx.enter_context(tc.tile_pool(name="consts", bufs=1))
        inp = ctx.enter_context(tc.tile_pool(name="inp", bufs=6))
        work = ctx.enter_context(tc.tile_pool(name="work", bufs=4))
        big = ctx.enter_context(tc.tile_pool(name="big", bufs=4))
        ps1 = ctx.enter_context(tc.tile_pool(name="ps1", bufs=1, space="PSUM"))
        ps2 = ctx.enter_context(tc.tile_pool(name="ps2", bufs=2, space="PSUM"))

        sb_blkw = consts.tile([128, 2 * NSPIN, 128], DT)
        nc.sync.dma_start(out=sb_blkw, in_=blkw.rearrange("s u k m -> k (s u) m"))
        sb_gqp = consts.tile([128, 4 * NSPIN, 128], DT)
        nc.sync.dma_start(out=sb_gqp, in_=gqp[:, :, :])
        sb_wsel = consts.tile([128, 4 * NSPIN, 32], DT)
        nc.sync.dma_start(out=sb_wsel, in_=wsel.rearrange("s q k m -> k (s q) m"))
        sb_borb = consts.tile([128, NSPIN], DT)
        nc.sync.dma_start(out=sb_borb, in_=borb[:, :])
        sb_i128 = consts.tile([128, 128], DT)
        nc.sync.dma_start(out=sb_i128, in_=i128[:, :])
        sb_eps = consts.tile([128, 1], DT)
        nc.vector.memset(sb_eps, 1e-12)

        loop_ctx = tc.For_i(0, repeat, 1) if hw_loop else None
        if loop_ctx is not None:
            ctx.enter_context(loop_ctx)
        for rep in range(1 if hw_loop else repeat):
          for gp in range(NPAIR):
            s = (2 * gp) // NWG

            sb_in = inp.tile([128, 160], DT)
            nc.sync.dma_start(out=sb_in, in_=ing[gp, :, :])
            xt2 = sb_in[:, 0:64]                         # [(b,f), (j, e)]
            res = sb_in[:, 64:160].rearrange("r (j i d) -> r j i d",
                                             j=2, d=3)

            # U,V block-diag matmuls -> one PSUM bank [128, (uv, j, o)]
            uv_ps = ps1.tile([128, 128], DT)
            nc.tensor.matmul(uv_ps[:, 0:64], sb_blkw[:, 2 * s + 0, :],
                             xt2, start=True, stop=True)
            nc.tensor.matmul(uv_ps[:, 64:128], sb_blkw[:, 2 * s + 1, :],
                             xt2, start=True, stop=True)
            # U' = U + b_orb (per-partition bias), PSUM -> SBUF; V copied
            # out too so the uv PSUM bank frees early (keeps PE unblocked)
            sb_u2 = work.tile([128, 64], DT)
            nc.scalar.activation(sb_u2, uv_ps[:, 0:64],
                                 mybir.ActivationFunctionType.Identity,
                                 bias=sb_borb[:, s:s + 1])
            sb_v2 = work.tile([128, 64], DT)
            nc.scalar.copy(sb_v2, uv_ps[:, 64:128])

            # resq products, layout [(b,i), (j, ion, dd8)], pads zeroed
            sb_resq = work.tile([128, 2, 16, 8], DT)
            nc.vector.memset(sb_resq[:, :, :, 6:8], 0.0)
            nc.vector.tensor_tensor(sb_resq[:, :, :, 0:3], res, res,
                                    op=mybir.AluOpType.mult)
            nc.vector.tensor_tensor(sb_resq[:, :, :, 3:5], res[:, :, :, 0:2],
                                    res[:, :, :, 1:3], op=mybir.AluOpType.mult)
            nc.vector.tensor_tensor(sb_resq[:, :, :, 5:6], res[:, :, :, 0:1],
                                    res[:, :, :, 2:3], op=mybir.AluOpType.mult)

            # full transpose per group -> rqt [(ion,dd8), (j, b*i)], base 0
            rqt_ps = ps1.tile([128, 2, 128], DT)
            for j in range(2):
                nc.tensor.transpose(
                    rqt_ps[:, j, :],
                    sb_resq[:, j].rearrange("r i d -> r (i d)"), sb_i128)
            sb_rqt = work.tile([128, 2, 128], DT)
            nc.scalar.copy(sb_rqt, rqt_ps)

            # dist2 (transposed): per-quad k=128 matmuls with zero-padded
            # lhsT -- out [(ionl,o), (q, j, b, i)], 2 PSUM banks
            d2_ps = ps2.tile([128, 4, 2, 128], DT)
            rqt_full = sb_rqt.rearrange("r j n -> r (j n)")
            for q in range(4):
                nc.tensor.matmul(
                    d2_ps[:, q].rearrange("r j n -> r (j n)"),
                    sb_gqp[:, 4 * s + q, :], rqt_full,
                    start=True, stop=True)
            # env chain in one ACT table set: dist=exp(0.5*ln(d2)); exp(-dist)
            sb_lnd = big.tile([128, 1024], DT)
            nc.scalar.activation(sb_lnd, d2_ps.rearrange("r q j n -> r (q j n)"),
                                 mybir.ActivationFunctionType.Ln, bias=sb_eps)
            sb_dist = big.tile([128, 1024], DT)
            nc.scalar.activation(sb_dist, sb_lnd,
                                 mybir.ActivationFunctionType.Exp, scale=0.5)
            sb_expd = big.tile([128, 4, 2, 128], DT)
            nc.scalar.activation(sb_expd.rearrange("r q j n -> r (q j n)"),
                                 sb_dist,
                                 mybir.ActivationFunctionType.Exp, scale=-1.0)

            # env: accumulate ion quads -> [32 o, (j, b, i)]
            env_ps = ps1.tile([32, 2, 128], DT)
            for q in range(4):
                nc.tensor.matmul(env_ps.rearrange("o j n -> o (j n)"),
                                 sb_wsel[:, 4 * s + q, :],
                                 sb_expd[:, q].rearrange("r j n -> r (j n)"),
                                 start=(q == 0), stop=(q == 3))
            sb_envo = work.tile([32, 2, 128], DT)
            nc.scalar.copy(sb_envo, env_ps)
            # [o, (j, b, i)] -> [(b,i), (j, o)] via PE transpose ...
            envn_ps = ps1.tile([128, 2, 32], DT)
            for j in range(2):
                nc.tensor.transpose(envn_ps[:, j, :], sb_envo[:, j, :],
                                    sb_i128[0:32, 0:32])
            sb_envn = work.tile([128, 2, 32], DT)
            nc.scalar.copy(sb_envn, envn_ps)
            # ... then 32x32 block transpose: [(b,i), (j,o)] -> [(b,o), (j,i)]
            sb_envt = work.tile([128, 2, 32], DT)
            if int(__import__("os").environ.get("NO_VT", "0")):
                nc.vector.tensor_copy(
                    sb_envt.rearrange("r j i -> r (j i)"),
                    sb_envn.rearrange("r j o -> r (j o)"))
            else:
                nc.vector.transpose(sb_envt.rearrange("r j i -> r (j i)"),
                                    sb_envn.rearrange("r j o -> r (j o)"))

            # Final two DVE passes over [128, (j, p, i)]:
            #   S = V bcast_p + U' bcast_i ;  out = S * env bcast_p
            v_pair = sb_v2.rearrange("r (j o) -> r j o", j=2)
            u_pair = sb_u2.rearrange("r (j o) -> r j o", j=2)
            v_b = v_pair[:, :, None, :].broadcast_to([128, 2, 32, 32])
            u_b = u_pair[:, :, :, None].broadcast_to([128, 2, 32, 32])
            env_b = sb_envt[:, :, None, :].broadcast_to([128, 2, 32, 32])
            sb_s = big.tile([128, 2, 32, 32], DT)
            nc.vector.tensor_tensor(sb_s, v_b, u_b, op=mybir.AluOpType.add)
            sb_out = big.tile([128, 2, 32, 32], DT)
            nc.vector.tensor_tensor(sb_out, sb_s, env_b,
                                    op=mybir.AluOpType.mult)
            nc.sync.dma_start(out=out[gp, :, :],
                              in_=sb_out.rearrange("r j p i -> r (j p i)"))

    nc.compile()
    return nc


def _host_constants(W_orb, b_orb, W_env_dim, w_env_ion):
    W_orb = np.asarray(W_orb, np.float32)
    b_orb = np.asarray(b_orb, np.float32)
    W_env_dim = np.asarray(W_env_dim, np.float32)
    w_env_ion = np.asarray(w_env_ion, np.float32)

    W1 = W_orb[:, :D, :]   # [s, f, o]
    W2 = W_orb[:, D:, :]

    blkw = np.zeros((NSPIN, 2, 128, 128), np.float32)
    for s in range(NSPIN):
        for bb in range(4):
            sl = slice(32 * bb, 32 * bb + 32)
            blkw[s, 0, sl, sl] = W1[s]
            blkw[s, 1, sl, sl] = W2[s]

    # G6[s, ion, k, o] with pair order _DD6, off-diagonal doubled
    G = np.einsum("siaoe,siboe->siabo", W_env_dim, W_env_dim)
    G6 = np.empty((NSPIN, NION, 6, NORB), np.float32)
    for k, (dA, dB) in enumerate(_DD6):
        G6[:, :, k, :] = G[:, :, dA, dB, :] * (1.0 if dA == dB else 2.0)

    gq = np.zeros((NSPIN, 4, 32, 128), np.float32)
    for s in range(NSPIN):
        for q in range(4):
            for il in range(4):
                gq[s, q, 8 * il:8 * il + 6, 32 * il:32 * il + 32] = \
                    G6[s, 4 * q + il]
    # gqp[(ion,dd8), (s,q), (ionl,o)]: quad-q block at rows 32q, zeros else
    gqp = np.zeros((128, 4 * NSPIN, 128), np.float32)
    for s in range(NSPIN):
        for q in range(4):
            gqp[32 * q:32 * q + 32, 4 * s + q, :] = gq[s, q]

    wsel = np.zeros((NSPIN, 4, 128, 32), np.float32)
    eye = np.eye(32, dtype=np.float32)
    for s in range(NSPIN):
        for q in range(4):
            for il in range(4):
                wsel[s, q, 32 * il:32 * il + 32, :] = \
                    w_env_ion[s, 4 * q + il] * eye

    borb = np.zeros((128, NSPIN), np.float32)
    for s in range(NSPIN):
        borb[:, s] = np.tile(b_orb[s], 4)

    i128 = np.eye(128, dtype=np.float32)
    return dict(blkw=blkw, gqp=gqp, wsel=wsel, borb=borb, i128=i128)


def _host_inputs(x, r_ei):
    x = np.asarray(x, np.float32)
    r_ei = np.asarray(r_ei, np.float32)
    xr = x.reshape(NCORES, NWG, 4, NELEC, D)
    rr = r_ei.reshape(NCORES, NWG, 4, NELEC, NION, SPATIAL)
    ing = np.empty((NCORES, NGRP, 128, 80), np.float32)
    for s in range(NSPIN):
        xs = xr[:, :, :, 32 * s:32 * s + 32, :]        # c,wg,b,e,f
        ing[:, NWG * s:NWG * (s + 1), :, 0:32] = \
            xs.transpose(0, 1, 2, 4, 3).reshape(NCORES, NWG, 128, 32)
        rs = rr[:, :, :, 32 * s:32 * s + 32, :, :]     # c,wg,b,i,ion,d
        ing[:, NWG * s:NWG * (s + 1), :, 32:80] = \
            rs.reshape(NCORES, NWG, 128, 48)
    # pack pairs: [xt(j0) | xt(j1) | res(j0) | res(j1)] -> [NPAIR, 128, 160]
    ing2 = np.concatenate([ing[:, 0::2, :, 0:32], ing[:, 1::2, :, 0:32],
                           ing[:, 0::2, :, 32:80], ing[:, 1::2, :, 32:80]],
                          axis=3)
    return np.ascontiguousarray(ing2)


def kernel(x, r_ei, W_orb, b_orb, W_env_dim, w_env_ion):
    global _NC_CACHE
    if _NC_CACHE is None:
        _NC_CACHE = _build_nc()
    nc = _NC_CACHE

    consts = _host_constants(W_orb, b_orb, W_env_dim, w_env_ion)
    ing = _host_inputs(x, r_ei)

    in_maps = [dict(ing=ing[c], **consts) for c in range(NCORES)]
    res = run_bass_kernel_spmd(nc, in_maps, core_ids=list(range(NCORES)))

    arr = np.stack([res.results[c]["out"] for c in range(NCORES)])
    # [c, pair, (b,o), (j,p,i)] ; pair = (s, pl), walker = (c, 2*pl+j, b)
    arr = arr.reshape(NCORES, NSPIN, NWG // 2, 4, 32, 2, 32, 32)
    # dims: c s pl b o j p i -> s c pl j b p i o
    out = arr.transpose(1, 0, 2, 5, 3, 6, 7, 4).reshape(
        NSPIN, B, NE, NE, NORB)
    return np.ascontiguousarray(out)


if __name__ == "__main__":
    rng = np.random.default_rng(0)
    x = rng.standard_normal((B, NELEC, D), dtype=np.float32)
    r_ei = rng.standard_normal((B, NELEC, NION, SPATIAL), dtype=np.float32)
    W_orb = rng.standard_normal((NSPIN, 2 * D, NORB), dtype=np.float32)
    b_orb = rng.standard_normal((NSPIN, NORB), dtype=np.float32)
    W_env_dim = rng.standard_normal((NSPIN, NION, SPATIAL, NORB, SPATIAL),
                                    dtype=np.float32)
    w_env_ion = rng.standard_normal((NSPIN, NION), dtype=np.float32)
    o = kernel(x=x, r_ei=r_ei, W_orb=W_orb, b_orb=b_orb,
               W_env_dim=W_env_dim, w_env_ion=w_env_ion)
    print(o.shape, o.dtype)
